# revision 27
# baseline (speedup 1.0000x reference)
"""Trainium2 Bass kernel for ChemicalNet (per-species MLP / MoE routing).

Strategy
--------
Only atoms whose species is in {1, 6, 7, 8} produce output (others are 0),
and each such atom only needs ITS OWN species' 3-layer MLP.  The reference
runs all 4 expert networks on all atoms; we route on the host instead:

- host: map species -> expert index, collect per-expert atom index lists
- shard: 2 cores per expert, each core gets half of that expert's atoms
  (the per-core in_map carries that expert's weights, so the single SPMD
  program is expert-agnostic)
- host passes the gathered embedding columns TRANSPOSED ([128, n]) so the
  device needs no transposes: PE contracts over the partition axis directly
- device: L1 matmul+SiLU, L2 matmul (2-step K accum)+SiLU, L3 matmul -> [1,n]
- host scatters the compact per-core outputs back to the full [N, 1] output

Performance notes (from NTFF traces of the fp32r version)
---------------------------------------------------------
- fp32r matmuls run fp32_mode=HIGH: ~2 cycles/col and no fast-weight-load.
  bf16 runs 1 col/cycle with FWL (4x faster LDWEIGHTS) and halves the
  embedding DMA.  Host-simulated bf16 end-to-end error is 4e-3 (threshold
  2e-2), so bf16 is the default.
- The PE HAM clock gate keeps the array at 1.2 GHz until it has been busy
  ~3.4us.  A burst of dummy warm-up matmuls at t=0 (while input DMAs run)
  moves the 2.4 GHz transition before the first real matmul.
- Input DMAs previously serialized ~5us on the sync HWDGE queue; emb chunks
  now alternate between the sync and scalar HWDGE queues.
- The scalar engine's ACTIVATE stream (SiLU) is the steady-state bottleneck
  (~1 elem/cycle/lane @ 1.2 GHz, dtype independent).

Per-chunk (512 atoms) the two 128-row halves of the hidden layer land in one
[128, 1024] PSUM tile so a single ACTIVATE applies SiLU to both.  That merge
needs a bias constant along the free axis; biases in this problem are
identically zero, which the host verifies -- nonzero-bias inputs take a
(slower) per-half ACTIVATE path with per-partition bias.

The layer-3 [1, F] matmul accumulates into a corner of the layer-2 PSUM
tile after its ACTIVATE has read it (WAR handled by Tile), so all 8 PSUM
banks go to the 4-deep [128, 1024] pipeline pool.

All shapes are compile-time constants derived from the actual input
(the Bass program is built fresh per call).
"""

import numpy as np

import concourse.bass as bass
import concourse.tile as tile
from concourse import bacc, mybir
from concourse.bass_utils import run_bass_kernel_spmd

N_CORES = 8
NSPECIES = 4
SPECIES_Z = np.array([1, 6, 7, 8], dtype=np.int32)
MAXIDX = 118
D = 128          # embedding dim
H = 256          # hidden dim
F = 512          # atom-chunk size (one PSUM bank of fp32)
FP = mybir.dt.float32
SILU = mybir.ActivationFunctionType.Silu
WARM_MMS = 9    # dummy matmuls at t=0 to trip the HAM clock gate early
WCOLS = 3 * H + 2  # combined weights tile: w1 [*,0:256], w2 [*,256:768], w3 [*,768:770]


def _build_program(npad: int, zero_bias: bool, mmdt, fold_l3: bool):
    """One SPMD program: a 3-layer per-expert MLP over `npad` atom columns.

    F=512 atom chunks ride a 4-deep rotation of [128, 1024] PSUM tiles
    (m-half 0 | m-half 1 along the free axis), which is the finest pipeline
    grain 8 PSUM banks allow: the PE runs L1 of chunk c+2 while the scalar
    engine (the bottleneck: SiLU at 1 elem/cycle/lane + ~352 cycles/instr
    tax) drains chunk c.  The ACTIVATE queue is software-pipelined
    (A1(c+2) sits between A2(c) and A2(c+1)) so the in-order scalar queue
    never head-of-line blocks.
    """
    nc = bacc.Bacc("TRN2", target_bir_lowering=False, debug=False,
                   num_devices=N_CORES)

    embT_d = nc.dram_tensor("embT", [D, npad], mmdt, kind="ExternalInput")
    w_d = nc.dram_tensor("w", [D, WCOLS], mmdt, kind="ExternalInput")
    if not zero_bias:
        b1_d = nc.dram_tensor("b1", [128, 2], FP, kind="ExternalInput")
        b2_d = nc.dram_tensor("b2", [128, 2], FP, kind="ExternalInput")
        b3_d = nc.dram_tensor("b3", [1, 1], FP, kind="ExternalInput")
    out_d = nc.dram_tensor("out", [1, npad], FP, kind="ExternalOutput")

    # ramped chunk sizes: small first chunks let the first ACTIVATEs start
    # while the bulk of the embedding is still streaming in
    sizes = []
    for s in (128, 256):
        if sum(sizes) + s <= npad:
            sizes.append(s)
    while npad - sum(sizes) > F:
        sizes.append(F)
    if npad - sum(sizes):
        sizes.append(npad - sum(sizes))
    chunks = []
    c0 = 0
    for s in sizes:
        chunks.append((c0, s))
        c0 += s
    nch = len(chunks)

    # Input DMA slabs.  Descriptor generation is the DMA bottleneck
    # (~90ns/descriptor/engine, 1 descriptor per partition per transfer,
    # serial per HWDGE ring), so the embedding streams in as a few wide
    # slabs, and the two transfers gating chunk 0 are tiny and lead their
    # rings: scalar ring [0:128], [128:384], s2; sync ring: weights, s1.
    b0 = min(384, npad)
    bounds = [0, b0]
    rest = npad - b0
    if rest > 0:
        mid = b0 + (rest + 3) // 4 * 2
        if mid < npad:
            bounds.append(mid)
        bounds.append(npad)
    slabs = [(a, b - a) for a, b in zip(bounds, bounds[1:])]
    # ring assignment: slab0 + the last slab scalar, the middle slab sync
    scalar_slabs = slabs[:1] + slabs[2:3]
    sync_slabs = slabs[1:2]

    with tile.TileContext(nc) as tc:
        with (
            tc.tile_pool(name="singles", bufs=1) as singles,
            tc.tile_pool(name="ps", bufs=1, space="PSUM") as psp,
        ):
            # All 8 PSUM banks as ONE tile, hand-rotated in four [128, 1024]
            # regions (Tile tracks dependencies per slice).  A single tile
            # means an ACTIVATE can span two address-adjacent regions, so
            # consecutive ACTIVATEs whose regions line up merge into one
            # instruction (the scalar engine pays ~352 cycles tax per
            # instruction; merging 4 pairs saves ~1.1us of the bottleneck).
            mega = psp.tile([128, 4096], FP)

            # --- t=0: warm the PE (HAM clock gate) with dummy matmuls on a
            # zeroed tile while the input DMAs stream in.  Disjoint 128-col
            # PSUM slices keep them independent (no WAW chain).
            warm_w = singles.tile([128, 128], mmdt)
            nc.vector.memset(warm_w[:], 0.0)
            for r in range(WARM_MMS):
                j = r % 8
                nc.tensor.matmul(mega[:, j * 128:(j + 1) * 128],
                                 warm_w[:], warm_w[:], start=True, stop=True)

            # preload the SiLU table set while input DMAs run (emitted
            # before the dma_starts: the ACT_TABLE_LOAD otherwise lands
            # between two scalar-ring slab transfers and delays the second)
            warm_act = singles.tile([128, 1], FP)
            nc.vector.memset(warm_act[:], 0.0)
            nc.scalar.activation(warm_act[:], warm_act[:], SILU)

            emb_t = singles.tile([D, npad], mmdt)
            out_t = singles.tile([1, npad], FP)
            w_t = singles.tile([D, WCOLS], mmdt)

            for s0, sw in scalar_slabs:
                nc.scalar.dma_start(emb_t[:, s0:s0 + sw],
                                    embT_d[:, s0:s0 + sw])
            nc.sync.dma_start(w_t[:], w_d[:])
            for s0, sw in sync_slabs:
                nc.sync.dma_start(emb_t[:, s0:s0 + sw], embT_d[:, s0:s0 + sw])

            if not zero_bias:
                b1_t = singles.tile([128, 2], FP)
                nc.gpsimd.dma_start(b1_t[:], b1_d[:])
                b2_t = singles.tile([128, 2], FP)
                nc.gpsimd.dma_start(b2_t[:], b2_d[:])
                b3_t = singles.tile([1, 1], FP)
                nc.gpsimd.dma_start(b3_t[:], b3_d[:])

            def m_off(f):
                # matmul output must stay inside one 512-col PSUM bank:
                # pack the m1 half right after m0 only when both fit bank 0
                return f if 2 * f <= F else F

            def extent(f):
                return m_off(f) + f

            # --- region rotation + pending-ACT batching -------------------
            # Each L1/L2 matmul group takes the next [128, 1024] region of
            # `mega` (round-robin, exactly the old 4-tile pool rotation).
            # Its ACTIVATE is held as `pending` until the next group is
            # emitted: if that group landed in the address-adjacent region
            # and the pending group fills (nearly) its whole region, ONE
            # ACTIVATE covers both.  ACT emission order is unchanged.
            state = {"reg": 1, "pending": None}  # region 0 went to warmup
            zview = {}   # (kind, ci) -> (z tile, column base)

            def emit_act(units):
                lo = units[0]["reg"] * 1024
                hi = units[-1]["reg"] * 1024 + extent(units[-1]["f"])
                z = singles.tile([128, hi - lo], mmdt,
                                 name="z_" + "_".join(
                                     f"{u['kind']}{u['ci']}" for u in units))
                nc.scalar.activation(z[:, :], mega[:, lo:hi], SILU)
                for u in units:
                    zview[(u["kind"], u["ci"])] = (z, u["reg"] * 1024 - lo)

            def emit_act_biased(u, b_t):
                # nonzero-bias fallback: per-m-half ACT with partition bias
                c0, f = chunks[u["ci"]]
                off = m_off(f)
                base = u["reg"] * 1024
                z = singles.tile([128, extent(f)], mmdt,
                                 name=f"z_{u['kind']}{u['ci']}")
                for m in range(2):
                    nc.scalar.activation(
                        z[:, m * off:m * off + f],
                        mega[:, base + m * off:base + m * off + f], SILU,
                        bias=b_t[:, m:m + 1])
                zview[(u["kind"], u["ci"])] = (z, 0)

            def push_group(kind, ci, f):
                u = {"kind": kind, "ci": ci, "f": f, "reg": state["reg"]}
                state["reg"] = (state["reg"] + 1) % 4
                p = state["pending"]
                if not zero_bias:
                    emit_act_biased(u, b1_t if kind == "L1" else b2_t)
                    return u["reg"]
                if p is not None:
                    if p["reg"] + 1 == u["reg"] and extent(p["f"]) >= 768:
                        emit_act([p, u])        # merged pair
                        state["pending"] = None
                    else:
                        emit_act([p])
                        state["pending"] = u
                else:
                    state["pending"] = u
                return u["reg"]

            def flush_pending():
                if state["pending"] is not None:
                    emit_act([state["pending"]])
                    state["pending"] = None

            l2reg = {}

            def emit_l1(ci):
                c0, f = chunks[ci]
                off = m_off(f)
                base = (state["reg"]) * 1024
                for m in range(2):
                    nc.tensor.matmul(
                        mega[:, base + m * off:base + m * off + f],
                        w_t[:, m * 128:(m + 1) * 128],
                        emb_t[:, c0:c0 + f], start=True, stop=True)
                push_group("L1", ci, f)

            def emit_l2(ci):
                c0, f = chunks[ci]
                z1, zb = zview[("L1", ci)]
                off = m_off(f)
                base = (state["reg"]) * 1024
                # m-half 1 first: the DVE copy of an earlier chunk's L3 row
                # may still be reading this region's bank-0 columns
                for m in (1, 0):
                    for k in range(2):
                        nc.tensor.matmul(
                            mega[:, base + m * off:base + m * off + f],
                            w_t[:, (1 + k) * H + m * 128:
                                (1 + k) * H + m * 128 + 128],
                            z1[:, zb + k * off:zb + k * off + f],
                            start=(k == 0), stop=(k == 1))
                l2reg[ci] = push_group("L2", ci, f)

            def emit_l3(ci):
                c0, f = chunks[ci]
                z2, zb = zview[("L2", ci)]
                off = m_off(f)
                base = l2reg[ci] * 1024
                # L3 accumulates into row 0 of the chunk's L2 region after
                # its ACT read (WAR handled by Tile); the region's next L2
                # overwrites it only after the copy below has read it.
                ps3 = mega[0:1, base:base + f]
                if fold_l3:
                    # out = w3b . (z2_m0*(w3a/w3b) + z2_m1): the combine
                    # runs on the (idle) DVE, halving L3's PE columns.
                    # Host sends w cols [768]=w3a/w3b, [769]=w3b.
                    v = singles.tile([128, f], mmdt, name=f"v_{ci}")
                    nc.vector.scalar_tensor_tensor(
                        v[:, :f], z2[:, zb:zb + f], w_t[:, 3 * H:3 * H + 1],
                        z2[:, zb + off:zb + off + f],
                        op0=mybir.AluOpType.mult, op1=mybir.AluOpType.add)
                    nc.tensor.matmul(ps3, w_t[:, 3 * H + 1:3 * H + 2],
                                     v[:, :f], start=True, stop=True)
                else:
                    nc.tensor.matmul(ps3, w_t[:, 3 * H:3 * H + 1],
                                     z2[:, zb:zb + f], start=True, stop=False)
                    nc.tensor.matmul(ps3, w_t[:, 3 * H + 1:3 * H + 2],
                                     z2[:, zb + off:zb + off + f],
                                     start=False, stop=True)
                if zero_bias:
                    nc.vector.tensor_copy(out_t[:, c0:c0 + f], ps3)
                else:
                    nc.vector.tensor_scalar_add(out_t[:, c0:c0 + f], ps3,
                                                b3_t[0:1, 0:1])

            depth = min(3, nch)
            for ci in range(depth):
                emit_l1(ci)
            for ci in range(nch):
                emit_l2(ci)
                if ci + depth < nch:
                    emit_l1(ci + depth)
                if ci >= 1:
                    emit_l3(ci - 1)
                    if ci == nch - 1:
                        # first span of the output ships while the last
                        # chunk finishes (out DMAs ride the idle sync ring)
                        bnd = chunks[ci - 1][0] + chunks[ci - 1][1]
                        nc.sync.dma_start(out_d[:, :bnd], out_t[:, :bnd])
            flush_pending()
            emit_l3(nch - 1)
            bnd = chunks[nch - 1][0] if nch > 1 else 0
            nc.sync.dma_start(out_d[:, bnd:npad], out_t[:, bnd:npad])

    nc.compile()
    return nc


def _route(species: np.ndarray):
    """species values -> expert idx (-1 unknown); per-core row assignments."""
    conv = np.full(MAXIDX + 2, -1, dtype=np.int32)
    conv[SPECIES_Z] = np.arange(NSPECIES, dtype=np.int32)
    idx = conv[species]
    core_rows = []
    for s in range(NSPECIES):
        rows = np.flatnonzero(idx == s)
        h = (len(rows) + 1) // 2
        core_rows.append(rows[:h])
        core_rows.append(rows[h:])
    return core_rows


def _run(inputs: dict, trace: bool = False, dtype_mode: str = "bf16"):
    species = inputs["species"]
    embedding = np.ascontiguousarray(inputs["embedding"], dtype=np.float32)
    n_atoms = species.shape[0]
    out_full = np.zeros((n_atoms, 1), dtype=np.float32)

    core_rows = _route(np.asarray(species))
    nmax = max(len(r) for r in core_rows)
    if nmax == 0:
        return out_full, None
    npad = -(-nmax // 8) * 8

    zero_bias = all(
        not np.any(np.asarray(inputs[k])) for k in ("b1", "b2", "b3"))
    mmdt = {"bf16": mybir.dt.bfloat16,
            "f32r": mybir.dt.float32r,
            "fp32": FP}[dtype_mode]
    np_mm = mybir.dt.np(mmdt)
    w3_all = np.asarray(inputs["W3"], dtype=np.float32).reshape(NSPECIES, 2, 128)
    # fold L3's K-halves through the DVE unless some w3b is too small for
    # the w3a/w3b ratio to be numerically safe (SPMD: one choice for all)
    fold_l3 = zero_bias and float(np.abs(w3_all[:, 1]).min()) > 1e-5
    nc = _build_program(npad, zero_bias, mmdt, fold_l3)

    in_maps = []
    for c in range(N_CORES):
        s = c // 2
        rows = core_rows[c]
        embT = np.zeros((D, npad), dtype=np_mm)
        if len(rows):
            embT[:, :len(rows)] = embedding[rows].T.astype(np_mm)
        # combined weights tile: [w1 | w2 row-half 0 | w2 row-half 1 | w3]
        w = np.zeros((D, WCOLS), dtype=np_mm)
        w[:, 0:H] = np.asarray(inputs["W1"][s], dtype=np.float32).astype(np_mm)
        w2 = np.asarray(inputs["W2"][s], dtype=np.float32).astype(np_mm)
        w[:, H:2 * H] = w2[0:128, :]
        w[:, 2 * H:3 * H] = w2[128:256, :]
        w3a, w3b = w3_all[s, 0], w3_all[s, 1]
        if fold_l3:
            w[:, 3 * H] = (w3a / w3b).astype(np_mm)
            w[:, 3 * H + 1] = w3b.astype(np_mm)
        else:
            w[:, 3 * H] = w3a.astype(np_mm)
            w[:, 3 * H + 1] = w3b.astype(np_mm)
        im = {
            "embT": embT,
            "w": np.ascontiguousarray(w),
        }
        if not zero_bias:
            im["b1"] = np.ascontiguousarray(
                np.asarray(inputs["b1"][s], dtype=np.float32).reshape(2, 128).T)
            im["b2"] = np.ascontiguousarray(
                np.asarray(inputs["b2"][s], dtype=np.float32).reshape(2, 128).T)
            im["b3"] = np.asarray(inputs["b3"][s], dtype=np.float32).reshape(1, 1)
        in_maps.append(im)

    res = run_bass_kernel_spmd(nc, in_maps, core_ids=list(range(N_CORES)),
                               trace=trace)
    for c in range(N_CORES):
        rows = core_rows[c]
        if len(rows):
            out_full[rows, 0] = res.results[c]["out"][0, :len(rows)]
    return out_full, res


def kernel(**inputs) -> np.ndarray:
    out, _ = _run(inputs, trace=False)
    return out


# revision 28
# speedup vs baseline: 1.1033x; 1.1033x over previous
"""Trainium2 Bass kernel for ChemicalNet (per-species MLP / MoE routing).

Strategy
--------
Only atoms whose species is in {1, 6, 7, 8} produce output (others are 0),
and each such atom only needs ITS OWN species' 3-layer MLP.  The reference
runs all 4 expert networks on all atoms; we route on the host instead:

- host: map species -> expert index, collect per-expert atom index lists
- shard: 2 cores per expert, each core gets half of that expert's atoms
  (the per-core in_map carries that expert's weights, so the single SPMD
  program is expert-agnostic)
- host passes the gathered embedding columns TRANSPOSED ([128, n]) so the
  device needs no transposes: PE contracts over the partition axis directly
- device: L1 matmul+SiLU, L2 matmul (2-step K accum)+SiLU, L3 matmul -> [1,n]
- host scatters the compact per-core outputs back to the full [N, 1] output

Performance notes (from NTFF traces of the fp32r version)
---------------------------------------------------------
- fp32r matmuls run fp32_mode=HIGH: ~2 cycles/col and no fast-weight-load.
  bf16 runs 1 col/cycle with FWL (4x faster LDWEIGHTS) and halves the
  embedding DMA.  Host-simulated bf16 end-to-end error is 4e-3 (threshold
  2e-2), so bf16 is the default.
- The PE HAM clock gate keeps the array at 1.2 GHz until it has been busy
  ~3.4us.  A burst of dummy warm-up matmuls at t=0 (while input DMAs run)
  moves the 2.4 GHz transition before the first real matmul.
- Input DMAs previously serialized ~5us on the sync HWDGE queue; emb chunks
  now alternate between the sync and scalar HWDGE queues.
- The scalar engine's ACTIVATE stream (SiLU) is the steady-state bottleneck
  (~1 elem/cycle/lane @ 1.2 GHz, dtype independent).

Per-chunk (512 atoms) the two 128-row halves of the hidden layer land in one
[128, 1024] PSUM tile so a single ACTIVATE applies SiLU to both.  That merge
needs a bias constant along the free axis; biases in this problem are
identically zero, which the host verifies -- nonzero-bias inputs take a
(slower) per-half ACTIVATE path with per-partition bias.

The layer-3 [1, F] matmul accumulates into a corner of the layer-2 PSUM
tile after its ACTIVATE has read it (WAR handled by Tile), so all 8 PSUM
banks go to the 4-deep [128, 1024] pipeline pool.

All shapes are compile-time constants derived from the actual input
(the Bass program is built fresh per call).
"""

import numpy as np

import concourse.bass as bass
import concourse.tile as tile
from concourse import bacc, mybir
from concourse.bass_utils import run_bass_kernel_spmd

N_CORES = 8
NSPECIES = 4
SPECIES_Z = np.array([1, 6, 7, 8], dtype=np.int32)
MAXIDX = 118
D = 128          # embedding dim
H = 256          # hidden dim
F = 512          # atom-chunk size (one PSUM bank of fp32)
FP = mybir.dt.float32
SILU = mybir.ActivationFunctionType.Silu
WARM_MMS = 9    # dummy matmuls at t=0 to trip the HAM clock gate early
WCOLS = 3 * H + 2  # combined weights tile: w1 [*,0:256], w2 [*,256:768], w3 [*,768:770]


def _build_program(npad: int, zero_bias: bool, mmdt):
    """One SPMD program: a 3-layer per-expert MLP over `npad` atom columns.

    F=512 atom chunks ride a 4-deep rotation of [128, 1024] PSUM tiles
    (m-half 0 | m-half 1 along the free axis), which is the finest pipeline
    grain 8 PSUM banks allow: the PE runs L1 of chunk c+2 while the scalar
    engine (the bottleneck: SiLU at 1 elem/cycle/lane + ~352 cycles/instr
    tax) drains chunk c.  The ACTIVATE queue is software-pipelined
    (A1(c+2) sits between A2(c) and A2(c+1)) so the in-order scalar queue
    never head-of-line blocks.
    """
    nc = bacc.Bacc("TRN2", target_bir_lowering=False, debug=False,
                   num_devices=N_CORES)

    embT_d = nc.dram_tensor("embT", [D, npad], mmdt, kind="ExternalInput")
    w_d = nc.dram_tensor("w", [D, WCOLS], mmdt, kind="ExternalInput")
    if not zero_bias:
        b1_d = nc.dram_tensor("b1", [128, 2], FP, kind="ExternalInput")
        b2_d = nc.dram_tensor("b2", [128, 2], FP, kind="ExternalInput")
        b3_d = nc.dram_tensor("b3", [1, 1], FP, kind="ExternalInput")
    out_d = nc.dram_tensor("out", [1, npad], FP, kind="ExternalOutput")

    # ramped chunk sizes: small first chunks let the first ACTIVATEs start
    # while the bulk of the embedding is still streaming in
    sizes = []
    for s in (128, 256):
        if sum(sizes) + s <= npad:
            sizes.append(s)
    while npad - sum(sizes) > F:
        sizes.append(F)
    if npad - sum(sizes):
        sizes.append(npad - sum(sizes))
    chunks = []
    c0 = 0
    for s in sizes:
        chunks.append((c0, s))
        c0 += s
    nch = len(chunks)

    # Input DMA slabs.  Descriptor generation is the DMA bottleneck
    # (~90ns/descriptor/engine, 1 descriptor per partition per transfer,
    # serial per HWDGE ring), so the embedding streams in as a few wide
    # slabs, and the two transfers gating chunk 0 are tiny and lead their
    # rings: scalar ring [0:128], [128:384], s2; sync ring: weights, s1.
    b0 = min(384, npad)
    bounds = [0, b0]
    rest = npad - b0
    if rest > 0:
        mid = b0 + (rest + 3) // 4 * 2
        if mid < npad:
            bounds.append(mid)
        bounds.append(npad)
    slabs = [(a, b - a) for a, b in zip(bounds, bounds[1:])]
    # ring assignment: slab0 + the last slab scalar, the middle slab sync
    scalar_slabs = slabs[:1] + slabs[2:3]
    sync_slabs = slabs[1:2]

    with tile.TileContext(nc) as tc:
        with (
            tc.tile_pool(name="singles", bufs=1) as singles,
            tc.tile_pool(name="ps", bufs=1, space="PSUM") as psp,
        ):
            # All 8 PSUM banks as ONE tile, hand-rotated in four [128, 1024]
            # regions (Tile tracks dependencies per slice).  A single tile
            # means an ACTIVATE can span two address-adjacent regions, so
            # consecutive ACTIVATEs whose regions line up merge into one
            # instruction (the scalar engine pays ~352 cycles tax per
            # instruction; merging 4 pairs saves ~1.1us of the bottleneck).
            mega = psp.tile([128, 4096], FP)

            # --- t=0: warm the PE (HAM clock gate) with dummy matmuls on a
            # zeroed tile while the input DMAs stream in.  Disjoint 128-col
            # PSUM slices keep them independent (no WAW chain).
            warm_w = singles.tile([128, 128], mmdt)
            nc.vector.memset(warm_w[:], 0.0)
            for r in range(WARM_MMS):
                j = r % 8
                nc.tensor.matmul(mega[:, j * 128:(j + 1) * 128],
                                 warm_w[:], warm_w[:], start=True, stop=True)

            # preload the SiLU table set while input DMAs run (emitted
            # before the dma_starts: the ACT_TABLE_LOAD otherwise lands
            # between two scalar-ring slab transfers and delays the second)
            warm_act = singles.tile([128, 1], FP)
            nc.vector.memset(warm_act[:], 0.0)
            nc.scalar.activation(warm_act[:], warm_act[:], SILU)

            emb_t = singles.tile([D, npad], mmdt)
            out_t = singles.tile([1, npad], FP)
            w_t = singles.tile([D, WCOLS], mmdt)

            for s0, sw in scalar_slabs:
                nc.scalar.dma_start(emb_t[:, s0:s0 + sw],
                                    embT_d[:, s0:s0 + sw])
            nc.sync.dma_start(w_t[:], w_d[:])
            for s0, sw in sync_slabs:
                nc.sync.dma_start(emb_t[:, s0:s0 + sw], embT_d[:, s0:s0 + sw])

            if not zero_bias:
                b1_t = singles.tile([128, 2], FP)
                nc.gpsimd.dma_start(b1_t[:], b1_d[:])
                b2_t = singles.tile([128, 2], FP)
                nc.gpsimd.dma_start(b2_t[:], b2_d[:])
                b3_t = singles.tile([1, 1], FP)
                nc.gpsimd.dma_start(b3_t[:], b3_d[:])

            def m_off(f):
                # matmul output must stay inside one 512-col PSUM bank:
                # pack the m1 half right after m0 only when both fit bank 0
                return f if 2 * f <= F else F

            def extent(f):
                return m_off(f) + f

            # --- region rotation + pending-ACT batching -------------------
            # Each L1/L2 matmul group takes the next [128, 1024] region of
            # `mega` (round-robin, exactly the old 4-tile pool rotation).
            # Its ACTIVATE is held as `pending` until the next group is
            # emitted: if that group landed in the address-adjacent region
            # and the pending group fills (nearly) its whole region, ONE
            # ACTIVATE covers both.  ACT emission order is unchanged.
            state = {"reg": 1, "pending": None}  # region 0 went to warmup
            zview = {}   # (kind, ci) -> (z tile, column base)

            def emit_act(units):
                lo = units[0]["reg"] * 1024
                hi = units[-1]["reg"] * 1024 + extent(units[-1]["f"])
                z = singles.tile([128, hi - lo], mmdt,
                                 name="z_" + "_".join(
                                     f"{u['kind']}{u['ci']}" for u in units))
                nc.scalar.activation(z[:, :], mega[:, lo:hi], SILU)
                for u in units:
                    zview[(u["kind"], u["ci"])] = (z, u["reg"] * 1024 - lo)

            def emit_act_biased(u, b_t):
                # nonzero-bias fallback: per-m-half ACT with partition bias
                c0, f = chunks[u["ci"]]
                off = m_off(f)
                base = u["reg"] * 1024
                z = singles.tile([128, extent(f)], mmdt,
                                 name=f"z_{u['kind']}{u['ci']}")
                for m in range(2):
                    nc.scalar.activation(
                        z[:, m * off:m * off + f],
                        mega[:, base + m * off:base + m * off + f], SILU,
                        bias=b_t[:, m:m + 1])
                zview[(u["kind"], u["ci"])] = (z, 0)

            def push_group(kind, ci, f):
                u = {"kind": kind, "ci": ci, "f": f, "reg": state["reg"]}
                state["reg"] = (state["reg"] + 1) % 4
                p = state["pending"]
                if not zero_bias:
                    emit_act_biased(u, b1_t if kind == "L1" else b2_t)
                    return u["reg"]
                if p is not None:
                    if p["reg"] + 1 == u["reg"] and extent(p["f"]) >= 768:
                        emit_act([p, u])        # merged pair
                        state["pending"] = None
                    else:
                        emit_act([p])
                        state["pending"] = u
                else:
                    state["pending"] = u
                return u["reg"]

            def flush_pending():
                if state["pending"] is not None:
                    emit_act([state["pending"]])
                    state["pending"] = None

            l2reg = {}

            def emit_l1(ci):
                c0, f = chunks[ci]
                off = m_off(f)
                base = (state["reg"]) * 1024
                for m in range(2):
                    nc.tensor.matmul(
                        mega[:, base + m * off:base + m * off + f],
                        w_t[:, m * 128:(m + 1) * 128],
                        emb_t[:, c0:c0 + f], start=True, stop=True)
                push_group("L1", ci, f)

            def emit_l2(ci):
                c0, f = chunks[ci]
                z1, zb = zview[("L1", ci)]
                off = m_off(f)
                base = (state["reg"]) * 1024
                # m-half 1 first: the DVE copy of an earlier chunk's L3 row
                # may still be reading this region's bank-0 columns
                for m in (1, 0):
                    for k in range(2):
                        nc.tensor.matmul(
                            mega[:, base + m * off:base + m * off + f],
                            w_t[:, (1 + k) * H + m * 128:
                                (1 + k) * H + m * 128 + 128],
                            z1[:, zb + k * off:zb + k * off + f],
                            start=(k == 0), stop=(k == 1))
                l2reg[ci] = push_group("L2", ci, f)

            def emit_l3(ci):
                c0, f = chunks[ci]
                z2, zb = zview[("L2", ci)]
                off = m_off(f)
                base = l2reg[ci] * 1024
                # L3 accumulates into row 0 of the chunk's L2 region after
                # its ACT read (WAR handled by Tile); the region's next L2
                # overwrites it only after the copy below has read it.
                ps3 = mega[0:1, base:base + f]
                nc.tensor.matmul(ps3, w_t[:, 3 * H:3 * H + 1],
                                 z2[:, zb:zb + f], start=True, stop=False)
                nc.tensor.matmul(ps3, w_t[:, 3 * H + 1:3 * H + 2],
                                 z2[:, zb + off:zb + off + f],
                                 start=False, stop=True)
                if zero_bias:
                    nc.vector.tensor_copy(out_t[:, c0:c0 + f], ps3)
                else:
                    nc.vector.tensor_scalar_add(out_t[:, c0:c0 + f], ps3,
                                                b3_t[0:1, 0:1])

            depth = min(3, nch)
            for ci in range(depth):
                emit_l1(ci)
            for ci in range(nch):
                emit_l2(ci)
                if ci + depth < nch:
                    emit_l1(ci + depth)
                if ci >= 1:
                    emit_l3(ci - 1)
                    if ci == nch - 1:
                        # first span of the output ships while the last
                        # chunk finishes (out DMAs ride the idle sync ring)
                        bnd = chunks[ci - 1][0] + chunks[ci - 1][1]
                        nc.sync.dma_start(out_d[:, :bnd], out_t[:, :bnd])
            flush_pending()
            emit_l3(nch - 1)
            bnd = chunks[nch - 1][0] if nch > 1 else 0
            nc.sync.dma_start(out_d[:, bnd:npad], out_t[:, bnd:npad])

    nc.compile()
    return nc


def _route(species: np.ndarray):
    """species values -> expert idx (-1 unknown); per-core row assignments."""
    conv = np.full(MAXIDX + 2, -1, dtype=np.int32)
    conv[SPECIES_Z] = np.arange(NSPECIES, dtype=np.int32)
    idx = conv[species]
    core_rows = []
    for s in range(NSPECIES):
        rows = np.flatnonzero(idx == s)
        h = (len(rows) + 1) // 2
        core_rows.append(rows[:h])
        core_rows.append(rows[h:])
    return core_rows


def _run(inputs: dict, trace: bool = False, dtype_mode: str = "bf16"):
    species = inputs["species"]
    embedding = np.ascontiguousarray(inputs["embedding"], dtype=np.float32)
    n_atoms = species.shape[0]
    out_full = np.zeros((n_atoms, 1), dtype=np.float32)

    core_rows = _route(np.asarray(species))
    nmax = max(len(r) for r in core_rows)
    if nmax == 0:
        return out_full, None
    npad = -(-nmax // 8) * 8

    zero_bias = all(
        not np.any(np.asarray(inputs[k])) for k in ("b1", "b2", "b3"))
    mmdt = {"bf16": mybir.dt.bfloat16,
            "f32r": mybir.dt.float32r,
            "fp32": FP}[dtype_mode]
    np_mm = mybir.dt.np(mmdt)
    nc = _build_program(npad, zero_bias, mmdt)

    in_maps = []
    for c in range(N_CORES):
        s = c // 2
        rows = core_rows[c]
        embT = np.zeros((D, npad), dtype=np_mm)
        if len(rows):
            embT[:, :len(rows)] = embedding[rows].T.astype(np_mm)
        # combined weights tile: [w1 | w2 row-half 0 | w2 row-half 1 | w3]
        w = np.zeros((D, WCOLS), dtype=np_mm)
        w[:, 0:H] = np.asarray(inputs["W1"][s], dtype=np.float32).astype(np_mm)
        w2 = np.asarray(inputs["W2"][s], dtype=np.float32).astype(np_mm)
        w[:, H:2 * H] = w2[0:128, :]
        w[:, 2 * H:3 * H] = w2[128:256, :]
        w[:, 3 * H:3 * H + 2] = np.asarray(
            inputs["W3"][s], dtype=np.float32).reshape(2, 128).T.astype(np_mm)
        im = {
            "embT": embT,
            "w": np.ascontiguousarray(w),
        }
        if not zero_bias:
            im["b1"] = np.ascontiguousarray(
                np.asarray(inputs["b1"][s], dtype=np.float32).reshape(2, 128).T)
            im["b2"] = np.ascontiguousarray(
                np.asarray(inputs["b2"][s], dtype=np.float32).reshape(2, 128).T)
            im["b3"] = np.asarray(inputs["b3"][s], dtype=np.float32).reshape(1, 1)
        in_maps.append(im)

    res = run_bass_kernel_spmd(nc, in_maps, core_ids=list(range(N_CORES)),
                               trace=trace)
    for c in range(N_CORES):
        rows = core_rows[c]
        if len(rows):
            out_full[rows, 0] = res.results[c]["out"][0, :len(rows)]
    return out_full, res


def kernel(**inputs) -> np.ndarray:
    out, _ = _run(inputs, trace=False)
    return out
